# revision 29
# baseline (speedup 1.0000x reference)
"""DigitCaps (capsule routing) forward pass on 8 TRN2 NeuronCores.

Data-parallel over the batch (8192 -> 1024/core). u_hat (si,50,29,8) is never
materialized. Per routing iteration:

  s[s,(j,a)]  = sum_{(i,b)} u[s,(i,b)] * (c[i,j] * Wmat[(i,b),(j,a)])   (matmul)
  G[(i,b),(j,a)] = sum_s u[s,(i,b)] * vj[s,(j,a)]
               = Weff_aug^T @ (x_aug^T @ vj)                            (2 matmuls)
  b_upd[i,j] = sum_{a,b} Wmat*G / si   -> tiny, AllGather'd across cores

The conv (50 filters 10x10 stride 5 on 20x20) is folded into a host-built
(401,450) matrix Weff_aug (row 400 = bias, paired with the ones row/column
the host appends to x). The host supplies x in BOTH layouts (batch-major,
padded to 408 cols with a ones column at 400, and pixel-major xT with a ones
row) so no on-chip transposes are needed.

The reps loop is software-pipelined: front(r+1) = x DMA + conv runs inside
AllGather#1(r)'s latency window, and iter0(r+1) (which depends only on conv,
never on a collective) runs inside AllGather#2(r)'s window.
"""

import numpy as np

import concourse.bacc as bacc
import concourse.mybir as mybir
import concourse.tile as tile
from concourse.bass_utils import run_bass_kernel_spmd

F32 = mybir.dt.float32
F32R = mybir.dt.float32r

N_CORES = 8
SI = 8192
B = SI // N_CORES  # 1024 per core
T = B // 128  # 8 batch tiles per core
XW = 408  # padded x row: 400 pixels + ones col + 7 pad
IC, IS = 50, 9  # in caps, in size
OC, OS = 29, 8  # out caps, out size
IB = IC * IS  # 450
JA = OC * OS  # 232
JAP = 256  # padded so fp32r matmul moving dim >= 256 (full PE rate)
QA = 401  # 400 pixels + 1 bias/ones row

# contraction chunks over q (pixels + ones row) and (i,b)
Q_CH = [(0, 128), (128, 128), (256, 128), (384, 17)]  # 16 pixels + ones/bias row
M_CH = [(0, 128), (128, 128), (256, 128), (384, 66)]
C0 = -float(np.log(OC))  # log_softmax of zeros


def _host_consts(W, conv_w, conv_b):
    """Build the small host-side constant matrices."""
    W = np.asarray(W, np.float32)
    conv_w = np.asarray(conv_w, np.float32).reshape(IC, 10, 10)
    conv_b = np.asarray(conv_b, np.float32)

    weff = np.zeros((QA, IB), np.float32)
    for oy in range(3):
        for ox in range(3):
            b = oy * 3 + ox
            for ky in range(10):
                for kx in range(10):
                    q = (5 * oy + ky) * 20 + (5 * ox + kx)
                    weff[q, np.arange(IC) * IS + b] = conv_w[:, ky, kx]
    weff[400, :] = np.repeat(conv_b, IS)  # bias row (paired with ones row of x^T)

    wmat = np.zeros((IB, JAP), np.float32)
    # Wmat[(i,b),(j,a)] = W[i,j,a,b]
    wmat[:, :JA] = W.transpose(0, 3, 1, 2).reshape(IB, JA)
    wc0 = (C0 * wmat).astype(np.float32)

    eind = np.zeros((IC, IB), np.float32)
    eind[np.arange(IB) // IS, np.arange(IB)] = 1.0
    return {
        "weff": weff,
        "wmat": wmat,
        "wc0": wc0,
        "eind": eind,
        "eindt": (eind.T / SI).copy(),  # 1/SI of the b-update folded in
    }


def build_nc(reps: int = 1, no_collective: bool = False, num_devices: int = N_CORES):
    nc = bacc.Bacc("TRN2", target_bir_lowering=False, debug=False, num_devices=num_devices)

    x_ext = nc.dram_tensor("x", [B, XW], F32R, kind="ExternalInput")
    xt_ext = nc.dram_tensor("xt", [QA, B], F32R, kind="ExternalInput")
    weff_ext = nc.dram_tensor("weff", [QA, IB], F32R, kind="ExternalInput")
    wmat_ext = nc.dram_tensor("wmat", [IB, JAP], F32, kind="ExternalInput")
    wc0_ext = nc.dram_tensor("wc0", [IB, JAP], F32R, kind="ExternalInput")
    eind_ext = nc.dram_tensor("eind", [IC, IB], F32R, kind="ExternalInput")
    eindt_ext = nc.dram_tensor("eindt", [IB, IC], F32, kind="ExternalInput")
    out_ext = nc.dram_tensor("out", [B, OC], F32, kind="ExternalOutput")

    A = mybir.ActivationFunctionType

    with tile.TileContext(nc) as tc:
        with (
            tc.tile_pool(name="const", bufs=1) as const,
            tc.tile_pool(name="xs", bufs=3) as xs_pool,
            tc.tile_pool(name="xts", bufs=2) as xts_pool,
            tc.tile_pool(name="ut", bufs=3) as ut_pool,
            tc.tile_pool(name="work", bufs=3) as work,
            tc.tile_pool(name="vjp", bufs=4) as vjp,
            tc.tile_pool(name="small", bufs=6) as small,
            tc.tile_pool(name="dram", bufs=4, space="DRAM") as dram,
            # one 4-deep pool of [128,512] banks shared by conv accumulators
            # and s accumulators (same tag -> same slots)
            tc.tile_pool(name="spsum", bufs=4, space="PSUM") as spsum,
        ):
            # ---- constants into SBUF ----
            b_sb = const.tile([IC, 32], F32, tag="btile")
            nc.vector.memset(b_sb[:], 0.0)  # once; pad cols stay 0 forever
            eps_sb = const.tile([128, 1], F32, tag="epstile")
            nc.vector.memset(eps_sb[:], 1e-30)

            weff012 = const.tile([128, 3 * IB], F32R, tag="weff012")
            nc.sync.dma_start(
                weff012[:].rearrange("p (c col) -> p c col", c=3),
                weff_ext[0:384, :].rearrange("(c p) col -> p c col", p=128),
            )
            weff3 = const.tile([17, IB], F32R, tag="weff3")
            nc.sync.dma_start(weff3[:], weff_ext[384:401, :])
            weff_c = [weff012[:, c * IB : (c + 1) * IB] for c in range(3)] + [weff3[:]]

            wc0012 = const.tile([128, 3 * JAP], F32R, tag="wc0012")
            nc.sync.dma_start(
                wc0012[:].rearrange("p (c col) -> p c col", c=3),
                wc0_ext[0:384, :].rearrange("(c p) col -> p c col", p=128),
            )
            wc03 = const.tile([66, JAP], F32R, tag="wc03")
            nc.sync.dma_start(wc03[:], wc0_ext[384:450, :])
            wc0_m = [wc0012[:, c * JAP : (c + 1) * JAP] for c in range(3)] + [wc03[:]]

            wmat012 = const.tile([128, 3 * JAP], F32, tag="wmat012")
            nc.sync.dma_start(
                wmat012[:].rearrange("p (c col) -> p c col", c=3),
                wmat_ext[0:384, :].rearrange("(c p) col -> p c col", p=128),
            )
            wmat3 = const.tile([66, JAP], F32, tag="wmat3")
            nc.sync.dma_start(wmat3[:], wmat_ext[384:450, :])
            wmat_m = [wmat012[:, c * JAP : (c + 1) * JAP] for c in range(3)] + [wmat3[:]]

            eindt012 = const.tile([128, 3 * IC], F32, tag="eindt012")
            nc.sync.dma_start(
                eindt012[:].rearrange("p (c col) -> p c col", c=3),
                eindt_ext[0:384, :].rearrange("(c p) col -> p c col", p=128),
            )
            eindt3 = const.tile([66, IC], F32, tag="eindt3")
            nc.sync.dma_start(eindt3[:], eindt_ext[384:450, :])
            eindt_m = [eindt012[:, c * IC : (c + 1) * IC] for c in range(3)] + [eindt3[:]]
            eind_sb = const.tile([IC, IB], F32R, tag="eind", name="eind")
            nc.gpsimd.dma_start(eind_sb[:], eind_ext[:])

            # ---------------- pipeline stages ----------------
            def front_dma():
                """x DMAs, issued ~a rep ahead of the conv that consumes them."""
                x_all = xs_pool.tile([128, T * XW], F32R, tag="xall")
                nc.sync.dma_start(
                    x_all[:].rearrange("p (t q) -> p t q", q=XW),
                    x_ext[:].rearrange("(t p) q -> p t q", p=128),
                )
                xt012 = xts_pool.tile([128, 3 * B], F32R, tag="xt012")
                nc.sync.dma_start(
                    xt012[:].rearrange("p (c col) -> p c col", c=3),
                    xt_ext[0:384, :].rearrange("(c p) col -> p c col", p=128),
                )
                xt3 = xts_pool.tile([17, B], F32R, tag="xt3")
                nc.sync.dma_start(xt3[:], xt_ext[384:401, :])
                return x_all, xt012, xt3

            def front_conv_half(d, h, uT=None):
                """One batch-half of the conv -> uT. Independent of all
                collectives; the two halves are emitted into different
                collective-wait windows."""
                x_all, xt012, xt3 = d
                xT = [xt012[:, c * B : (c + 1) * B] for c in range(3)] + [xt3[:]]
                if uT is None:
                    uT = []
                    for ms, mn in M_CH:
                        uT.append(
                            ut_pool.tile([mn, B], F32R, tag=f"uT{ms}", name=f"uT{ms}")
                        )
                for m, (ms, mn) in enumerate(M_CH):
                    pu = spsum.tile([128, 512], F32, tag="sps")
                    for c, (qs, qn) in enumerate(Q_CH):
                        nc.tensor.matmul(
                            pu[:mn, :],
                            weff_c[c][:, ms : ms + mn],
                            xT[c][:, h * 512 : (h + 1) * 512],
                            start=(c == 0),
                            stop=(c == 3),
                        )
                    eng = nc.scalar if m % 2 == 0 else nc.vector
                    if eng is nc.scalar:
                        eng.copy(uT[m][:, h * 512 : (h + 1) * 512], pu[:mn, :])
                    else:
                        eng.tensor_copy(uT[m][:, h * 512 : (h + 1) * 512], pu[:mn, :])
                return uT

            def mk_state(d, uT):
                x_all = d[0]
                xt_b = [x_all[:, t * XW : t * XW + 400] for t in range(T)]
                xh_b = [x_all[:, t * XW + 384 : t * XW + 401] for t in range(T)]
                return dict(uT=uT, xt=xt_b, xh=xh_b)

            def front_conv(d, uT_h0=None):
                uT = front_conv_half(d, 0) if uT_h0 is None else uT_h0
                front_conv_half(d, 1, uT)
                return mk_state(d, uT)

            def coeffs(it):
                """log-softmax of b_sb (rows are tiny: |b|<1, no max-sub
                needed) -> per-chunk c-weighted wmat tiles."""
                e_t = small.tile([IC, OC], F32, tag="et")
                z = small.tile([IC, 1], F32, tag="z")
                nc.scalar.activation(e_t[:], b_sb[:, 0:OC], A.Exp, accum_out=z[:])
                lz = small.tile([IC, 1], F32, tag="lz")
                nc.scalar.activation(lz[:], z[:], A.Ln)
                c_sb = work.tile([IC, 32], F32R, tag="csb")
                nc.vector.scalar_tensor_tensor(
                    c_sb[:],
                    b_sb[:],
                    1.0,
                    lz[:].to_broadcast([IC, 32]),
                    op0=mybir.AluOpType.mult,
                    op1=mybir.AluOpType.subtract,
                )
                wc_t = []
                with tc.tile_pool(name=f"cbps{it}", bufs=2, space="PSUM") as cb_pool:
                    for m, (ms, mn) in enumerate(M_CH):
                        cb = cb_pool.tile([128, 32], F32, tag="cb", name="cb")
                        nc.tensor.matmul(
                            cb[0:mn, :],
                            eind_sb[:, ms : ms + mn],
                            c_sb[:],
                            start=True,
                            stop=True,
                        )
                        w = work.tile([128, JAP], F32R, tag=f"wc{ms}", name=f"wc{ms}")
                        nc.vector.tensor_mul(
                            w[0:mn, :].rearrange("p (j a) -> p j a", a=OS),
                            wmat_m[m][:].rearrange("p (j a) -> p j a", a=OS),
                            cb[0:mn, :].unsqueeze(-1).to_broadcast([mn, 32, OS]),
                        )
                        wc_t.append(w)
                return wc_t

            def iter_mid(st, wc_t, key):
                """One non-final routing iteration: s -> squash -> vj -> H/G
                -> local b-update -> AllGather trigger. Returns the handles
                the post-collective step needs."""
                uT, xt, xh = st["uT"], st["xt"], st["xh"]
                hctx = tc.tile_pool(name=f"hps{key}", bufs=1, space="PSUM")
                hps_pool = hctx.__enter__()
                h_ps = [
                    hps_pool.tile([128, JAP], F32, tag="h0", name="h0"),
                    hps_pool.tile([128, JAP], F32, tag="h1", name="h1"),
                    hps_pool.tile([128, JAP], F32, tag="h2", name="h2"),
                    hps_pool.tile([17, JAP], F32, tag="h3", name="h3"),
                ]
                for tp in range(T // 2):
                    s_ps = spsum.tile([128, 2 * JAP], F32, tag="sps")
                    for half in range(2):
                        t = 2 * tp + half
                        for kc, (ms, mn) in enumerate(M_CH):
                            nc.tensor.matmul(
                                s_ps[:, half * JAP : (half + 1) * JAP],
                                uT[kc][:, t * 128 : (t + 1) * 128],
                                wc_t[kc][0:mn, :],
                                start=(kc == 0),
                                stop=(kc == 3),
                                skip_group_check=True,
                            )
                    sq = work.tile([128, 2 * JAP], F32, tag="sq")
                    nc.scalar.activation(sq[:], s_ps[:], A.Square)
                    ssum = small.tile([128, 64], F32, tag="ssum")
                    nc.vector.reduce_sum(
                        ssum[:],
                        sq[:].rearrange("p (j a) -> p j a", a=OS),
                        axis=mybir.AxisListType.X,
                    )
                    lnv = small.tile([128, 64], F32, tag="lnv")
                    nc.scalar.activation(lnv[:], ssum[:], A.Ln, bias=eps_sb[:])
                    lnp = small.tile([128, 64], F32, tag="lnp")
                    nc.scalar.activation(lnp[:], ssum[:], A.Ln, bias=1.0)
                    dln = small.tile([128, 64], F32, tag="dln")
                    nc.vector.scalar_tensor_tensor(
                        dln[:],
                        lnv[:],
                        0.5,
                        lnp[:],
                        op0=mybir.AluOpType.mult,
                        op1=mybir.AluOpType.subtract,
                    )
                    scl = small.tile([128, 64], F32, tag="scl")
                    nc.scalar.activation(scl[:], dln[:], A.Exp)
                    vj = vjp.tile([128, 2 * JAP], F32R, tag="vj")
                    nc.vector.tensor_mul(
                        vj[:].rearrange("p (g a) -> p g a", a=OS),
                        s_ps[:].rearrange("p (g a) -> p g a", a=OS),
                        scl[:].unsqueeze(-1).to_broadcast([128, 64, OS]),
                    )
                    for half in range(2):
                        t = 2 * tp + half
                        vjh = vj[:, half * JAP : (half + 1) * JAP]
                        for c, (qs, qn) in enumerate(Q_CH):
                            lhs = xt[t][:, qs : qs + qn] if c < 3 else xh[t]
                            nc.tensor.matmul(
                                h_ps[c][0:qn, :],
                                lhs,
                                vjh,
                                start=(t == 0),
                                stop=(t == T - 1),
                                skip_group_check=True,
                            )

                # -- H -> sbuf, G, agreement --
                hs_sb = work.tile([128, 2 * JAP], F32R, tag="hsA")
                nc.scalar.copy(hs_sb[:, 0:JAP], h_ps[0][:])
                nc.vector.tensor_copy(hs_sb[:, JAP : 2 * JAP], h_ps[1][:])
                hs_sb2 = work.tile([128, 2 * JAP], F32R, tag="hsB")
                nc.scalar.copy(hs_sb2[:, 0:JAP], h_ps[2][:])
                nc.vector.tensor_copy(hs_sb2[0:17, JAP : 2 * JAP], h_ps[3][:])
                hs = [
                    hs_sb[:, 0:JAP],
                    hs_sb[:, JAP : 2 * JAP],
                    hs_sb2[:, 0:JAP],
                    hs_sb2[0:17, JAP : 2 * JAP],
                ]
                hctx.__exit__(None, None, None)

                ag_in = dram.tile([IC, OC], F32, tag="agin")
                ag_out = dram.tile(
                    [N_CORES * IC, OC], F32, addr_space="Shared", tag="agout"
                )
                with tc.tile_pool(name=f"gps{key}", bufs=1, space="PSUM") as gps_pool:
                    # per m-chunk: G matmuls -> wmat*G -> grouped reduce ->
                    # bps accumulate, so DVE pipelines behind PE.
                    # three separate loops: bps matmuls would otherwise
                    # head-of-line block the next m's G matmuls on the
                    # in-order PE queue while DVE computes pm/rm
                    g_all = gps_pool.tile([128, 4 * JAP], F32, tag="gall")
                    bps = gps_pool.tile([IC, OC], F32, tag="bps", name="bps")
                    for m, (ms, mn) in enumerate(M_CH):
                        for c in range(4):
                            nc.tensor.matmul(
                                g_all[0:mn, m * JAP : (m + 1) * JAP],
                                weff_c[c][:, ms : ms + mn],
                                hs[c][:],
                                start=(c == 0),
                                stop=(c == 3),
                                skip_group_check=True,
                            )
                    rms = []
                    for m, (ms, mn) in enumerate(M_CH):
                        pm = work.tile([128, JA], F32, tag="pm")
                        nc.vector.tensor_mul(
                            pm[0:mn, :],
                            wmat_m[m][0:mn, 0:JA],
                            g_all[0:mn, m * JAP : m * JAP + JA],
                        )
                        rm = work.tile([128, OC], F32, tag="rm")
                        nc.vector.reduce_sum(
                            rm[0:mn, :],
                            pm[0:mn, :].rearrange("p (j a) -> p j a", a=OS),
                            axis=mybir.AxisListType.X,
                        )
                        rms.append(rm)
                    for m, (ms, mn) in enumerate(M_CH):
                        nc.tensor.matmul(
                            bps[:],
                            eindt_m[m][:],
                            rms[m][0:mn, :],
                            start=(m == 0),
                            stop=(m == 3),
                            skip_group_check=True,
                        )
                    bu = work.tile([IC, OC], F32, tag="bu")
                    nc.scalar.copy(bu[:], bps[:])  # 1/SI pre-folded into eindt

                nc.scalar.dma_start(ag_in[:], bu[:])
                if not no_collective:
                    nc.gpsimd.collective_compute(
                        "AllGather",
                        mybir.AluOpType.bypass,
                        ins=[ag_in[:]],
                        outs=[ag_out[:]],
                        replica_groups=[list(range(N_CORES))],
                    )
                return ag_in, ag_out

            def ag_post(it, ag_in, ag_out):
                """Collective result -> b_sb update (b_sb += sum over cores)."""
                agg = work.tile([IC, N_CORES * OC], F32, tag="agg")
                if no_collective:
                    nc.sync.dma_start(
                        agg[:].rearrange("i (r j) -> i r j", r=N_CORES),
                        ag_in[:].unsqueeze(1).to_broadcast([IC, N_CORES, OC]),
                    )
                else:
                    nc.sync.dma_start(
                        agg[:].rearrange("i (r j) -> i r j", r=N_CORES),
                        ag_out[:].rearrange("(r i) j -> i r j", i=IC),
                    )
                a1 = work.tile([IC, 4 * OC], F32, tag="a1")
                nc.vector.tensor_add(a1[:], agg[:, 0 : 4 * OC], agg[:, 4 * OC : 8 * OC])
                a2 = work.tile([IC, 2 * OC], F32, tag="a2")
                nc.vector.tensor_add(a2[:], a1[:, 0 : 2 * OC], a1[:, 2 * OC : 4 * OC])
                if it == 0:
                    nc.vector.tensor_add(
                        b_sb[:, 0:OC], a2[:, 0:OC], a2[:, OC : 2 * OC]
                    )
                else:
                    upd = work.tile([IC, OC], F32, tag="upd")
                    nc.vector.tensor_add(upd[:], a2[:, 0:OC], a2[:, OC : 2 * OC])
                    nc.vector.tensor_add(b_sb[:, 0:OC], b_sb[:, 0:OC], upd[:])

            def iter_last(st, wc_t):
                uT = st["uT"]
                ov_all = work.tile([128, T * 32], F32, tag="ovall")
                ssum_all = work.tile([128, T * 32], F32, tag="ssall")
                for tp in range(T // 2):
                    s_ps = spsum.tile([128, 2 * JAP], F32, tag="sps")
                    for half in range(2):
                        t = 2 * tp + half
                        for kc, (ms, mn) in enumerate(M_CH):
                            nc.tensor.matmul(
                                s_ps[:, half * JAP : (half + 1) * JAP],
                                uT[kc][:, t * 128 : (t + 1) * 128],
                                wc_t[kc][0:mn, :],
                                start=(kc == 0),
                                stop=(kc == 3),
                                skip_group_check=True,
                            )
                    sq = work.tile([128, 2 * JAP], F32, tag="sq")
                    nc.scalar.activation(sq[:], s_ps[:], A.Square)
                    nc.vector.reduce_sum(
                        ssum_all[:, tp * 64 : (tp + 1) * 64],
                        sq[:].rearrange("p (j a) -> p j a", a=OS),
                        axis=mybir.AxisListType.X,
                    )
                lnv = work.tile([128, T * 32], F32, tag="lnva")
                nc.scalar.activation(lnv[:], ssum_all[:], A.Ln, bias=eps_sb[:])
                nc.scalar.activation(ov_all[:], lnv[:], A.Exp, scale=0.5)
                for tp in range(T // 2):
                    nc.sync.dma_start(
                        out_ext[:]
                        .rearrange("(t p) j -> p t j", p=128)[:, 2 * tp : 2 * tp + 2, :],
                        ov_all[:, tp * 64 : (tp + 1) * 64]
                        .rearrange("p (t j) -> p t j", j=32)[:, :, 0:OC],
                    )

            # ---------------- software-pipelined reps loop ----------------
            dmas = {0: front_dma()}
            if reps > 1:
                dmas[1] = front_dma()
            st = front_conv(dmas.pop(0))
            ag1 = iter_mid(st, wc0_m, "i0r0")
            nxt = front_conv(dmas.pop(1)) if reps > 1 else None
            for r in range(reps):
                ag_post(0, *ag1)
                wc_t = coeffs(f"1r{r}")
                ag2 = iter_mid(st, wc_t, f"i1r{r}")
                if r + 2 < reps:
                    dmas[r + 2] = front_dma()
                if r + 1 < reps:
                    nxt_ag1 = iter_mid(nxt, wc0_m, f"i0r{r+1}")
                ag_post(1, *ag2)
                wc_t = coeffs(f"2r{r}")
                uT2 = front_conv_half(dmas[r + 2], 0) if r + 2 < reps else None
                iter_last(st, wc_t)
                if r + 1 < reps:
                    st, ag1 = nxt, nxt_ag1
                    nxt = (
                        front_conv(dmas.pop(r + 2), uT2) if r + 2 < reps else None
                    )

    nc.compile()
    _dedupe_act_table_loads(nc)
    return nc


def _dedupe_act_table_loads(nc):
    """bacc's set picker alternates exp_and_others(0) / natural_log(5) for
    our Exp+Ln mix. Every function we use (Exp, Ln, Square, Identity, Copy)
    is in natural_log_exp_and_others (id 6), so one load suffices."""
    from concourse.hw_specs import get_activation_tables

    tabs = list(get_activation_tables(nc.m.arch).items())
    target = next(i for i, (nm, _) in enumerate(tabs) if nm == "natural_log_exp_and_others")
    used = {
        i.func
        for b in nc.main_func.blocks
        for i in b.instructions
        if type(i).__name__ == "InstActivation"
    }
    assert used <= tabs[target][1], (used, tabs[target][1])
    first = True
    for b in nc.main_func.blocks:
        kept = []
        for i in b.instructions:
            if type(i).__name__ == "InstLoadActFuncSet":
                si = i.sync_info
                if first:
                    i.act_func_set_id = target
                    first = False
                    kept.append(i)
                    continue
                if si is not None and (len(si.on_wait) or len(si.on_update)):
                    # keep any load carrying sync duties, just retarget it
                    i.act_func_set_id = target
                    kept.append(i)
                continue
            kept.append(i)
        b.instructions[:] = kept


_NC_CACHE = {}


def _get_nc(reps: int = 1, **kw):
    key = (reps, tuple(sorted(kw.items())))
    if key not in _NC_CACHE:
        _NC_CACHE[key] = build_nc(reps, **kw)
    return _NC_CACHE[key]


def make_in_maps(x, W, conv_w, conv_b):
    consts = _host_consts(W, conv_w, conv_b)
    x = np.asarray(x, np.float32)
    in_maps = []
    for i in range(N_CORES):
        xs = x[i * B : (i + 1) * B]
        xp = np.zeros((B, XW), np.float32)
        xp[:, :400] = xs
        xp[:, 400] = 1.0
        xtp = np.empty((QA, B), np.float32)
        xtp[:400] = xs.T
        xtp[400] = 1.0
        m = {"x": xp, "xt": np.ascontiguousarray(xtp)}
        m.update(consts)
        in_maps.append(m)
    return in_maps


def kernel(x, W, conv_w, conv_b, _trace=False):
    nc = _get_nc()
    in_maps = make_in_maps(x, W, conv_w, conv_b)
    r = run_bass_kernel_spmd(
        nc, in_maps, list(range(N_CORES)), trace=_trace
    )
    out = np.concatenate([r.results[i]["out"] for i in range(N_CORES)], axis=0)
    kernel.last_results = r
    return out.astype(np.float32)


# revision 43
# speedup vs baseline: 1.1582x; 1.1582x over previous
"""DigitCaps (capsule routing) forward pass on 8 TRN2 NeuronCores.

Data-parallel over the batch (8192 -> 1024/core). u_hat (si,50,29,8) is never
materialized, and neither is u: per routing iteration, s is computed by
reassociating

  s = (x_aug @ Weff) @ (c o Wmat)  =  x_aug @ [ Weff @ (c o Wmat) ]
                                              `-- WV: (401,256), 16 matmuls --'

so the conv never runs as a standalone pass. WV for iteration 0 (c = const
log-softmax of zeros) is host-precomputed. The agreement side stays

  G[(i,b),(j,a)] = Weff_aug^T @ (x_aug^T @ vj);  db = sum_{a,b} Wmat*G / si

with db AllGather'd + summed across cores each iteration. The host supplies
x in both layouts (batch-major padded to 408 cols with a ones column, and
pixel-major x^T with a ones row).

The reps loop is software-pipelined: iter0(r+1) (which depends on no
collective at all) fills AllGather#2(r)'s latency window; x DMAs are issued
a rep ahead.
"""

import numpy as np

import concourse.bacc as bacc
import concourse.mybir as mybir
import concourse.tile as tile
from concourse.bass_utils import run_bass_kernel_spmd

F32 = mybir.dt.float32
F32R = mybir.dt.float32r

N_CORES = 8
SI = 8192
B = SI // N_CORES  # 1024 per core
T = B // 128  # 8 batch tiles per core
XW = 408  # padded x row: 400 pixels + ones col + 7 pad
IC, IS = 50, 9  # in caps, in size
OC, OS = 29, 8  # out caps, out size
IB = IC * IS  # 450
JA = OC * OS  # 232
JAP = 256  # padded so fp32r matmul moving dim >= 256 (full PE rate)
QA = 401  # 400 pixels + 1 bias/ones row

# contraction chunks over q (pixels + ones row) and (i,b)
Q_CH = [(0, 128), (128, 128), (256, 128), (384, 17)]  # 16 pixels + ones/bias row
M_CH = [(0, 128), (128, 128), (256, 128), (384, 66)]
C0 = -float(np.log(OC))  # log_softmax of zeros


def _host_consts(W, conv_w, conv_b):
    """Build the small host-side constant matrices."""
    W = np.asarray(W, np.float32)
    conv_w = np.asarray(conv_w, np.float32).reshape(IC, 10, 10)
    conv_b = np.asarray(conv_b, np.float32)

    weff = np.zeros((QA, IB), np.float32)
    for oy in range(3):
        for ox in range(3):
            b = oy * 3 + ox
            for ky in range(10):
                for kx in range(10):
                    q = (5 * oy + ky) * 20 + (5 * ox + kx)
                    weff[q, np.arange(IC) * IS + b] = conv_w[:, ky, kx]
    weff[400, :] = np.repeat(conv_b, IS)  # bias row (paired with ones row of x^T)

    wmat = np.zeros((IB, JAP), np.float32)
    # Wmat[(i,b),(j,a)] = W[i,j,a,b]
    wmat[:, :JA] = W.transpose(0, 3, 1, 2).reshape(IB, JA)
    wv0 = (C0 * (weff @ wmat)).astype(np.float32)  # (401, 256)

    eind = np.zeros((IC, IB), np.float32)
    eind[np.arange(IB) // IS, np.arange(IB)] = 1.0
    return {
        "weff": weff,
        "wefft": np.ascontiguousarray(weff.T),
        "wmat": wmat,
        "wv0": wv0,
        "eind": eind,
        "eindt": (eind.T / SI).copy(),  # 1/SI of the b-update folded in
    }


def build_nc(reps: int = 1, no_collective: bool = False, num_devices: int = N_CORES):
    nc = bacc.Bacc("TRN2", target_bir_lowering=False, debug=False, num_devices=num_devices)

    x_ext = nc.dram_tensor("x", [B, XW], F32R, kind="ExternalInput")
    xt_ext = nc.dram_tensor("xt", [QA, B], F32R, kind="ExternalInput")
    weff_ext = nc.dram_tensor("weff", [QA, IB], F32R, kind="ExternalInput")
    wefft_ext = nc.dram_tensor("wefft", [IB, QA], F32R, kind="ExternalInput")
    wmat_ext = nc.dram_tensor("wmat", [IB, JAP], F32, kind="ExternalInput")
    wv0_ext = nc.dram_tensor("wv0", [QA, JAP], F32R, kind="ExternalInput")
    eind_ext = nc.dram_tensor("eind", [IC, IB], F32R, kind="ExternalInput")
    eindt_ext = nc.dram_tensor("eindt", [IB, IC], F32, kind="ExternalInput")
    out_ext = nc.dram_tensor("out", [B, OC], F32, kind="ExternalOutput")

    A = mybir.ActivationFunctionType

    with tile.TileContext(nc) as tc:
        with (
            tc.tile_pool(name="const", bufs=1) as const,
            tc.tile_pool(name="xs", bufs=4) as xs_pool,
            tc.tile_pool(name="xts", bufs=4) as xts_pool,
            tc.tile_pool(name="work", bufs=3) as work,
            tc.tile_pool(name="wk2", bufs=2) as wk2,
            tc.tile_pool(name="vjp", bufs=3) as vjp,
            tc.tile_pool(name="small", bufs=5) as small,
            tc.tile_pool(name="dram", bufs=4, space="DRAM") as dram,
            # 4-deep pool of [128,512] PSUM banks shared by s accumulators
            # and the per-iteration WV accumulators (same tag -> same slots)
            tc.tile_pool(name="spsum", bufs=4, space="PSUM") as spsum,
        ):
            # ---- constants into SBUF ----
            b_sb = const.tile([IC, 32], F32, tag="btile")
            nc.vector.memset(b_sb[:], 0.0)  # once; pad cols stay 0 forever
            bn_sb = const.tile([IC, 32], F32, tag="bntile")
            nc.vector.memset(bn_sb[:], 0.0)
            eps_sb = const.tile([128, 1], F32, tag="epstile")
            nc.vector.memset(eps_sb[:], 1e-30)

            weff012 = const.tile([128, 3 * IB], F32R, tag="weff012")
            nc.sync.dma_start(
                weff012[:].rearrange("p (c col) -> p c col", c=3),
                weff_ext[0:384, :].rearrange("(c p) col -> p c col", p=128),
            )
            weff3 = const.tile([17, IB], F32R, tag="weff3")
            nc.sync.dma_start(weff3[:], weff_ext[384:401, :])
            weff_c = [weff012[:, c * IB : (c + 1) * IB] for c in range(3)] + [weff3[:]]

            wefft012 = const.tile([128, 3 * QA], F32R, tag="wefft012")
            nc.sync.dma_start(
                wefft012[:].rearrange("p (c col) -> p c col", c=3),
                wefft_ext[0:384, :].rearrange("(c p) col -> p c col", p=128),
            )
            wefft3 = const.tile([66, QA], F32R, tag="wefft3")
            nc.sync.dma_start(wefft3[:], wefft_ext[384:450, :])
            wefft_m = [wefft012[:, c * QA : (c + 1) * QA] for c in range(3)] + [
                wefft3[:]
            ]

            wv0012 = const.tile([128, 3 * JAP], F32R, tag="wv0012")
            nc.sync.dma_start(
                wv0012[:].rearrange("p (c col) -> p c col", c=3),
                wv0_ext[0:384, :].rearrange("(c p) col -> p c col", p=128),
            )
            wv03 = const.tile([17, JAP], F32R, tag="wv03")
            nc.sync.dma_start(wv03[:], wv0_ext[384:401, :])
            wv0_q = [wv0012[:, c * JAP : (c + 1) * JAP] for c in range(3)] + [wv03[:]]

            wmat012 = const.tile([128, 3 * JAP], F32, tag="wmat012")
            nc.sync.dma_start(
                wmat012[:].rearrange("p (c col) -> p c col", c=3),
                wmat_ext[0:384, :].rearrange("(c p) col -> p c col", p=128),
            )
            wmat3 = const.tile([66, JAP], F32, tag="wmat3")
            nc.sync.dma_start(wmat3[:], wmat_ext[384:450, :])
            wmat_m = [wmat012[:, c * JAP : (c + 1) * JAP] for c in range(3)] + [wmat3[:]]

            eindt012 = const.tile([128, 3 * IC], F32, tag="eindt012")
            nc.sync.dma_start(
                eindt012[:].rearrange("p (c col) -> p c col", c=3),
                eindt_ext[0:384, :].rearrange("(c p) col -> p c col", p=128),
            )
            eindt3 = const.tile([66, IC], F32, tag="eindt3")
            nc.sync.dma_start(eindt3[:], eindt_ext[384:450, :])
            eindt_m = [eindt012[:, c * IC : (c + 1) * IC] for c in range(3)] + [eindt3[:]]
            eind_sb = const.tile([IC, IB], F32R, tag="eind", name="eind")
            nc.gpsimd.dma_start(eind_sb[:], eind_ext[:])

            # ---------------- pipeline stages ----------------
            def front_dma():
                """x DMAs, issued ~a rep ahead of first use."""
                x_all = xs_pool.tile([128, T * XW], F32R, tag="xall")
                nc.sync.dma_start(
                    x_all[:].rearrange("p (t q) -> p t q", q=XW),
                    x_ext[:].rearrange("(t p) q -> p t q", p=128),
                )
                xt012 = xts_pool.tile([128, 3 * B], F32R, tag="xt012")
                nc.sync.dma_start(
                    xt012[:].rearrange("p (c col) -> p c col", c=3),
                    xt_ext[0:384, :].rearrange("(c p) col -> p c col", p=128),
                )
                xt3 = xts_pool.tile([17, B], F32R, tag="xt3")
                nc.sync.dma_start(xt3[:], xt_ext[384:401, :])
                xT = [xt012[:, c * B : (c + 1) * B] for c in range(3)] + [xt3[:]]
                xt_b = [x_all[:, t * XW : t * XW + 400] for t in range(T)]
                xh_b = [x_all[:, t * XW + 384 : t * XW + 401] for t in range(T)]
                return dict(xT=xT, xt=xt_b, xh=xh_b)

            def coeffs(it, b_src):
                """log-softmax of b_src (rows are tiny: |b|<1, no max-sub
                needed) -> c-weighted Wmat -> WV = Weff @ (c o Wmat)."""
                e_t = small.tile([IC, OC], F32, tag="et")
                z = small.tile([IC, 1], F32, tag="z")
                nc.scalar.activation(e_t[:], b_src[:, 0:OC], A.Exp, accum_out=z[:])
                lz = small.tile([IC, 1], F32, tag="lz")
                nc.scalar.activation(lz[:], z[:], A.Ln)
                c_sb = work.tile([IC, 32], F32R, tag="csb")
                nc.vector.scalar_tensor_tensor(
                    c_sb[:],
                    b_src[:],
                    1.0,
                    lz[:].to_broadcast([IC, 32]),
                    op0=mybir.AluOpType.mult,
                    op1=mybir.AluOpType.subtract,
                )
                wc_t = []
                with tc.tile_pool(name=f"cbps{it}", bufs=2, space="PSUM") as cb_pool:
                    for m, (ms, mn) in enumerate(M_CH):
                        cb = cb_pool.tile([128, 32], F32, tag="cb", name="cb")
                        nc.tensor.matmul(
                            cb[0:mn, :],
                            eind_sb[:, ms : ms + mn],
                            c_sb[:],
                            start=True,
                            stop=True,
                        )
                        w = work.tile([128, JAP], F32R, tag=f"wc{ms}", name=f"wc{ms}")
                        nc.vector.tensor_mul(
                            w[0:mn, :].rearrange("p (j a) -> p j a", a=OS),
                            wmat_m[m][:].rearrange("p (j a) -> p j a", a=OS),
                            cb[0:mn, :].unsqueeze(-1).to_broadcast([mn, 32, OS]),
                        )
                        wc_t.append(w)
                # WV = Weff_aug @ (c o Wmat): (401, 256) in two PSUM banks,
                # two q-chunks per bank (sequential accumulation groups)
                wv_sb = []
                for bank in range(2):
                    wv_ps = spsum.tile([128, 2 * JAP], F32, tag="sps")
                    for half in range(2):
                        qs, qn = Q_CH[2 * bank + half]
                        for kc, (ms, mn) in enumerate(M_CH):
                            nc.tensor.matmul(
                                wv_ps[0:qn, half * JAP : (half + 1) * JAP],
                                wefft_m[kc][:, qs : qs + qn],
                                wc_t[kc][0:mn, :],
                                start=(kc == 0),
                                stop=(kc == 3),
                                skip_group_check=True,
                            )
                    sb = wk2.tile([128, 2 * JAP], F32R, tag=f"wv{bank}")
                    if bank == 0:
                        nc.scalar.copy(sb[:], wv_ps[:])
                    else:
                        nc.scalar.copy(sb[:, 0:JAP], wv_ps[:, 0:JAP])
                        nc.vector.tensor_copy(
                            sb[0:17, JAP : 2 * JAP], wv_ps[0:17, JAP : 2 * JAP]
                        )
                    wv_sb.append(sb)
                return [
                    wv_sb[0][:, 0:JAP],
                    wv_sb[0][:, JAP : 2 * JAP],
                    wv_sb[1][:, 0:JAP],
                    wv_sb[1][0:17, JAP : 2 * JAP],
                ]

            def s_matmuls(st, wv_q, s_ps, t):
                """s[t-tile] = x_aug[t] @ WV : accumulate over 4 q-chunks."""
                half = t % 2
                for qc, (qs, qn) in enumerate(Q_CH):
                    nc.tensor.matmul(
                        s_ps[:, half * JAP : (half + 1) * JAP],
                        st["xT"][qc][:, t * 128 : (t + 1) * 128],
                        wv_q[qc][0:qn, :],
                        start=(qc == 0),
                        stop=(qc == 3),
                        skip_group_check=True,
                    )

            def iter_mid(st, wv_q, key, ag_in, slot, dup=False):
                """One non-final routing iteration: s -> squash -> vj -> H/G
                -> local b-update written into ag_in rows [slot*IC:(slot+1)*IC]
                (no collective here — the caller batches two iterations'
                updates into one AllGather)."""
                xt, xh = st["xt"], st["xh"]
                hctx = tc.tile_pool(name=f"hps{key}", bufs=1, space="PSUM")
                hps_pool = hctx.__enter__()
                h_ps = [
                    hps_pool.tile([128, JAP], F32, tag="h0", name="h0"),
                    hps_pool.tile([128, JAP], F32, tag="h1", name="h1"),
                    hps_pool.tile([128, JAP], F32, tag="h2", name="h2"),
                    hps_pool.tile([17, JAP], F32, tag="h3", name="h3"),
                ]
                for tp in range(T // 2):
                    s_ps = spsum.tile([128, 2 * JAP], F32, tag="sps")
                    for half in range(2):
                        s_matmuls(st, wv_q, s_ps, 2 * tp + half)
                    sq = wk2.tile([128, 2 * JAP], F32, tag="sq")
                    nc.scalar.activation(sq[:], s_ps[:], A.Square)
                    ssum = small.tile([128, 64], F32, tag="ssum")
                    nc.vector.reduce_sum(
                        ssum[:],
                        sq[:].rearrange("p (j a) -> p j a", a=OS),
                        axis=mybir.AxisListType.X,
                    )
                    lnv = small.tile([128, 64], F32, tag="lnv")
                    nc.scalar.activation(lnv[:], ssum[:], A.Ln, bias=eps_sb[:])
                    lnp = small.tile([128, 64], F32, tag="lnp")
                    nc.scalar.activation(lnp[:], ssum[:], A.Ln, bias=1.0)
                    dln = small.tile([128, 64], F32, tag="dln")
                    nc.vector.scalar_tensor_tensor(
                        dln[:],
                        lnv[:],
                        0.5,
                        lnp[:],
                        op0=mybir.AluOpType.mult,
                        op1=mybir.AluOpType.subtract,
                    )
                    scl = small.tile([128, 64], F32, tag="scl")
                    nc.scalar.activation(scl[:], dln[:], A.Exp)
                    vj = vjp.tile([128, 2 * JAP], F32R, tag="vj")
                    nc.vector.tensor_mul(
                        vj[:].rearrange("p (g a) -> p g a", a=OS),
                        s_ps[:].rearrange("p (g a) -> p g a", a=OS),
                        scl[:].unsqueeze(-1).to_broadcast([128, 64, OS]),
                    )
                    for half in range(2):
                        t = 2 * tp + half
                        vjh = vj[:, half * JAP : (half + 1) * JAP]
                        for c, (qs, qn) in enumerate(Q_CH):
                            lhs = xt[t][:, qs : qs + qn] if c < 3 else xh[t]
                            nc.tensor.matmul(
                                h_ps[c][0:qn, :],
                                lhs,
                                vjh,
                                start=(t == 0),
                                stop=(t == T - 1),
                                skip_group_check=True,
                            )

                # -- H -> sbuf, G, agreement --
                hs_sb = wk2.tile([128, 2 * JAP], F32R, tag="hsA")
                nc.scalar.copy(hs_sb[:, 0:JAP], h_ps[0][:])
                nc.vector.tensor_copy(hs_sb[:, JAP : 2 * JAP], h_ps[1][:])
                hs_sb2 = wk2.tile([128, 2 * JAP], F32R, tag="hsB")
                nc.scalar.copy(hs_sb2[:, 0:JAP], h_ps[2][:])
                nc.vector.tensor_copy(hs_sb2[0:17, JAP : 2 * JAP], h_ps[3][:])
                hs = [
                    hs_sb[:, 0:JAP],
                    hs_sb[:, JAP : 2 * JAP],
                    hs_sb2[:, 0:JAP],
                    hs_sb2[0:17, JAP : 2 * JAP],
                ]
                hctx.__exit__(None, None, None)

                with tc.tile_pool(name=f"gps{key}", bufs=1, space="PSUM") as gps_pool:
                    # three separate loops: bps matmuls would otherwise
                    # head-of-line block the next m's G matmuls on the
                    # in-order PE queue while DVE computes pm/rm
                    g_all = gps_pool.tile([128, 4 * JAP], F32, tag="gall")
                    bps = gps_pool.tile([IC, OC], F32, tag="bps", name="bps")
                    for m, (ms, mn) in enumerate(M_CH):
                        for c in range(4):
                            nc.tensor.matmul(
                                g_all[0:mn, m * JAP : (m + 1) * JAP],
                                weff_c[c][:, ms : ms + mn],
                                hs[c][:],
                                start=(c == 0),
                                stop=(c == 3),
                                skip_group_check=True,
                            )
                    rms = []
                    for m, (ms, mn) in enumerate(M_CH):
                        pm = work.tile([128, JA], F32, tag="pm")
                        nc.vector.tensor_mul(
                            pm[0:mn, :],
                            wmat_m[m][0:mn, 0:JA],
                            g_all[0:mn, m * JAP : m * JAP + JA],
                        )
                        rm = work.tile([128, OC], F32, tag="rm")
                        nc.vector.reduce_sum(
                            rm[0:mn, :],
                            pm[0:mn, :].rearrange("p (j a) -> p j a", a=OS),
                            axis=mybir.AxisListType.X,
                        )
                        rms.append(rm)
                    for m, (ms, mn) in enumerate(M_CH):
                        nc.tensor.matmul(
                            bps[:],
                            eindt_m[m][:],
                            rms[m][0:mn, :],
                            start=(m == 0),
                            stop=(m == 3),
                            skip_group_check=True,
                        )
                    bu = work.tile([IC, OC], F32, tag="bu")
                    nc.scalar.copy(bu[:], bps[:])  # 1/SI pre-folded into eindt

                nc.scalar.dma_start(ag_in[:, slot * OC : (slot + 1) * OC], bu[:])
                if dup:  # keep the unused slot finite for the full-width adds
                    nc.scalar.dma_start(
                        ag_in[:, (1 - slot) * OC : (2 - slot) * OC], bu[:]
                    )

            def mk_ag():
                ag_in = dram.tile([IC, 2 * OC], F32, tag="agin")
                ag_out = dram.tile(
                    [N_CORES * IC, 2 * OC], F32, addr_space="Shared", tag="agout"
                )
                return ag_in, ag_out

            def ag_fire(ag_in, ag_out):
                if not no_collective:
                    nc.gpsimd.collective_compute(
                        "AllGather",
                        mybir.AluOpType.bypass,
                        ins=[ag_in[:]],
                        outs=[ag_out[:]],
                        replica_groups=[list(range(N_CORES))],
                    )

            def ag_post(ag_in, ag_out, have_b2, have_b1):
                """One fused collective result -> cross-core sums.
                cols 0:OC    = this rep's iter1 update  -> b_sb = bn_sb + sum
                cols OC:2OC  = next rep's iter0 update  -> bn_sb = sum"""
                J2 = 2 * OC
                agg = work.tile([IC, N_CORES * J2], F32, tag="agg")
                if no_collective:
                    nc.sync.dma_start(
                        agg[:].rearrange("i (r j) -> i r j", r=N_CORES),
                        ag_in[:].unsqueeze(1).to_broadcast([IC, N_CORES, J2]),
                    )
                else:
                    nc.sync.dma_start(
                        agg[:].rearrange("i (r j) -> i r j", r=N_CORES),
                        ag_out[:].rearrange("(r i) j -> i r j", i=IC),
                    )
                a1 = work.tile([IC, 4 * J2], F32, tag="a1")
                nc.vector.tensor_add(a1[:], agg[:, 0 : 4 * J2], agg[:, 4 * J2 : 8 * J2])
                a2 = work.tile([IC, 2 * J2], F32, tag="a2")
                nc.vector.tensor_add(a2[:], a1[:, 0 : 2 * J2], a1[:, 2 * J2 : 4 * J2])
                upd = work.tile([IC, J2], F32, tag="upd")
                nc.vector.tensor_add(upd[:], a2[:, 0:J2], a2[:, J2 : 2 * J2])
                if have_b2:  # b2 of this rep = b1 (in bn_sb) + summed update
                    nc.vector.tensor_add(b_sb[:, 0:OC], bn_sb[:, 0:OC], upd[:, 0:OC])
                if have_b1:  # b1 of the next rep (after the read above)
                    nc.vector.tensor_copy(bn_sb[:, 0:OC], upd[:, OC:J2])

            def iter_last(st, wv_q):
                ov_all = work.tile([128, T * 32], F32, tag="ovall")
                ssum_all = work.tile([128, T * 32], F32, tag="ssall")
                for tp in range(T // 2):
                    s_ps = spsum.tile([128, 2 * JAP], F32, tag="sps")
                    for half in range(2):
                        s_matmuls(st, wv_q, s_ps, 2 * tp + half)
                    sq = wk2.tile([128, 2 * JAP], F32, tag="sq")
                    nc.scalar.activation(sq[:], s_ps[:], A.Square)
                    nc.vector.reduce_sum(
                        ssum_all[:, tp * 64 : (tp + 1) * 64],
                        sq[:].rearrange("p (j a) -> p j a", a=OS),
                        axis=mybir.AxisListType.X,
                    )
                lnv = work.tile([128, T * 32], F32, tag="lnva")
                nc.scalar.activation(lnv[:], ssum_all[:], A.Ln, bias=eps_sb[:])
                nc.scalar.activation(ov_all[:], lnv[:], A.Exp, scale=0.5)
                for tp in range(T // 2):
                    nc.sync.dma_start(
                        out_ext[:]
                        .rearrange("(t p) j -> p t j", p=128)[:, 2 * tp : 2 * tp + 2, :],
                        ov_all[:, tp * 64 : (tp + 1) * 64]
                        .rearrange("p (t j) -> p t j", j=32)[:, :, 0:OC],
                    )

            # ---------------- software-pipelined reps loop ----------------
            # One fused AllGather per rep: slot0 carries iter1(r)'s update,
            # slot1 carries iter0(r+1)'s, which was computed a whole rep
            # earlier (iter0 runs two reps ahead, right after the previous
            # rep's fire, where it hides the collective's latency).
            sts = {0: front_dma()}
            if reps > 1:
                sts[1] = front_dma()
            agp = mk_ag()
            iter_mid(sts[0], wv0_q, "i0r0", agp[0], slot=1, dup=True)
            ag_fire(*agp)
            agcs = {0: mk_ag()}
            if reps > 1:
                iter_mid(sts[1], wv0_q, "i0r1", agcs[0][0], slot=1)
            if reps > 2:
                sts[2] = front_dma()
            ag_post(*agp, have_b2=False, have_b1=True)  # bn = b1(0)
            for r in range(reps):
                wv_q = coeffs(f"1r{r}", bn_sb)
                iter_mid(
                    sts[r], wv_q, f"i1r{r}", agcs[r][0], slot=0, dup=(r + 1 == reps)
                )
                ag_fire(*agcs[r])
                if r + 1 < reps:
                    agcs[r + 1] = mk_ag()
                    if r + 2 < reps:
                        iter_mid(
                            sts[r + 2], wv0_q, f"i0r{r+2}", agcs[r + 1][0], slot=1
                        )
                        if r + 3 < reps:
                            sts[r + 3] = front_dma()
                ag_post(*agcs[r], have_b2=True, have_b1=(r + 1 < reps))
                agcs.pop(r)
                wv_q = coeffs(f"2r{r}", b_sb)
                iter_last(sts.pop(r), wv_q)

    nc.compile()
    _dedupe_act_table_loads(nc)
    return nc


def _dedupe_act_table_loads(nc):
    """bacc's set picker alternates exp_and_others(0) / natural_log(5) for
    our Exp+Ln mix. Every function we use (Exp, Ln, Square, Identity, Copy)
    is in natural_log_exp_and_others (id 6), so one load suffices."""
    from concourse.hw_specs import get_activation_tables

    tabs = list(get_activation_tables(nc.m.arch).items())
    target = next(i for i, (nm, _) in enumerate(tabs) if nm == "natural_log_exp_and_others")
    used = {
        i.func
        for b in nc.main_func.blocks
        for i in b.instructions
        if type(i).__name__ == "InstActivation"
    }
    assert used <= tabs[target][1], (used, tabs[target][1])
    first = True
    for b in nc.main_func.blocks:
        kept = []
        for i in b.instructions:
            if type(i).__name__ == "InstLoadActFuncSet":
                si = i.sync_info
                if first:
                    i.act_func_set_id = target
                    first = False
                    kept.append(i)
                    continue
                if si is not None and (len(si.on_wait) or len(si.on_update)):
                    # keep any load carrying sync duties, just retarget it
                    i.act_func_set_id = target
                    kept.append(i)
                continue
            kept.append(i)
        b.instructions[:] = kept


_NC_CACHE = {}


def _get_nc(reps: int = 1, **kw):
    key = (reps, tuple(sorted(kw.items())))
    if key not in _NC_CACHE:
        _NC_CACHE[key] = build_nc(reps, **kw)
    return _NC_CACHE[key]


def make_in_maps(x, W, conv_w, conv_b):
    consts = _host_consts(W, conv_w, conv_b)
    x = np.asarray(x, np.float32)
    in_maps = []
    for i in range(N_CORES):
        xs = x[i * B : (i + 1) * B]
        xp = np.zeros((B, XW), np.float32)
        xp[:, :400] = xs
        xp[:, 400] = 1.0
        xtp = np.empty((QA, B), np.float32)
        xtp[:400] = xs.T
        xtp[400] = 1.0
        m = {"x": xp, "xt": np.ascontiguousarray(xtp)}
        m.update(consts)
        in_maps.append(m)
    return in_maps


def kernel(x, W, conv_w, conv_b, _trace=False):
    nc = _get_nc()
    in_maps = make_in_maps(x, W, conv_w, conv_b)
    r = run_bass_kernel_spmd(
        nc, in_maps, list(range(N_CORES)), trace=_trace
    )
    out = np.concatenate([r.results[i]["out"] for i in range(N_CORES)], axis=0)
    kernel.last_results = r
    return out.astype(np.float32)
